# revision 10
# baseline (speedup 1.0000x reference)
"""MinGRU Trainium2 kernel.

Problem: nn_MinGRU (B=8, T=4096, D=1024, fp32)
    k       = h @ W_z.T + b_z
    th      = h @ W_h.T + b_h
    log-space parallel scan of  h[t] = (1-z[t]) * h[t-1] + z[t]*g(th[t])
with z = sigmoid(k), g(x) = x+0.5 for x>=0 else sigmoid(x).

We compute the mathematically identical linear-space recurrence directly:
    a[t] = sigmoid(-k[t])          (= 1 - z)
    b[t] = sigmoid(k[t]) * g(th[t]),   g(x) = max(x + 0.5, sigmoid(x))
    h[t] = a[t] * h[t-1] + b[t]    (VectorE tensor_tensor_scan, fp32 state)
This is numerically stable since a in (0,1), so it matches the reference's
log-space computation to fp32 rounding.

Sharding: data-parallel over batch — core i gets sample i ([T, D]).
Weights replicated, pre-transposed on host to [d, e] layout (matmul lhsT).

Per-core layout: channels e on partitions, time t along the free dim so the
scan runs along free. h is loaded with a casting DMA (fp32->bf16) and
transposed to [d, t] via the DMA xbar; matmuls run in bf16 (fp32 PSUM
accumulate); everything after the matmul is fp32. Output is transposed back
to [t, e] with PE transposes.
"""

import numpy as np
import concourse.bass as bass
import concourse.bacc as bacc
import concourse.mybir as mybir
import concourse.tile as tile
from concourse.bass_utils import run_bass_kernel_spmd

F32 = mybir.dt.float32
BF16 = mybir.dt.bfloat16
AF = mybir.ActivationFunctionType
OP = mybir.AluOpType

B, T, D = 8, 4096, 1024
NC_CORES = 8
TC = 512                 # time chunk (free dim of one PSUM bank in fp32)
NCHUNK = T // TC         # 8
NE = D // 128            # 8 e-tiles (output channel blocks)
ND = D // 128            # 8 d-tiles (contraction blocks)
NTB = TC // 128          # 4 t-blocks per chunk


def build_program():
    nc = bacc.Bacc("TRN2", target_bir_lowering=False, debug=False)
    h_d = nc.dram_tensor("h", [T, D], F32, kind="ExternalInput").ap()
    wzT_d = nc.dram_tensor("wzT", [D, D], F32, kind="ExternalInput").ap()
    whT_d = nc.dram_tensor("whT", [D, D], F32, kind="ExternalInput").ap()
    bz_d = nc.dram_tensor("bz", [128, NE], F32, kind="ExternalInput").ap()
    bh_d = nc.dram_tensor("bh", [128, NE], F32, kind="ExternalInput").ap()
    out_d = nc.dram_tensor("out", [T, D], F32, kind="ExternalOutput").ap()

    with tile.TileContext(nc) as tc:
        import contextlib
        with contextlib.ExitStack() as ctx:
            const = ctx.enter_context(tc.tile_pool(name="const", bufs=1))
            hnatp = ctx.enter_context(tc.tile_pool(name="hnat", bufs=2))
            hTp = ctx.enter_context(tc.tile_pool(name="hT", bufs=2))
            mmps = ctx.enter_context(tc.tile_pool(name="mmps", bufs=4, space="PSUM"))
            ew = ctx.enter_context(tc.tile_pool(name="ew", bufs=2))
            hscp = ctx.enter_context(tc.tile_pool(name="hsc", bufs=2))
            osbp = ctx.enter_context(tc.tile_pool(name="osb", bufs=2))

            # ---- constants ----
            wz_sb = const.tile([128, ND, D], BF16)   # [d_in_tile, d_tile, e]
            wh_sb = const.tile([128, ND, D], BF16)
            # cast fp32->bf16 during DMA (SWDGE)
            nc.gpsimd.dma_start(wz_sb, wzT_d.rearrange("(dt p) e -> p dt e", p=128))
            nc.gpsimd.dma_start(wh_sb, whT_d.rearrange("(dt p) e -> p dt e", p=128))
            bz_sb = const.tile([128, NE], F32)
            bh_sb = const.tile([128, NE], F32)
            nc.sync.dma_start(bz_sb, bz_d)
            nc.sync.dma_start(bh_sb, bh_d)
            negbz = const.tile([128, NE], F32)
            bh05 = const.tile([128, NE], F32)
            nc.gpsimd.tensor_scalar_mul(negbz, bz_sb, -1.0)
            nc.gpsimd.tensor_scalar_add(bh05, bh_sb, 0.5)

            prev_h = [None] * NE

            for tci in range(NCHUNK):
                # ---- load h chunk (cast to bf16), natural [t, d] layout ----
                h_nat = hnatp.tile([128, NTB, D], BF16, name=f"h_nat{tci}",
                                   tag="h_nat")
                src = bass.AP(
                    tensor=h_d.tensor,
                    offset=h_d.offset + tci * TC * D,
                    ap=[[D, 128], [128 * D, NTB], [1, D]],
                )
                nc.gpsimd.dma_start(h_nat, src)

                # ---- transpose to [d, t] via DMA xbar (bf16, no engine cost) ----
                hT = hTp.tile([128, ND, TC], BF16, name=f"hT{tci}", tag="hT")
                for tb in range(NTB):
                    nc.sync.dma_start(
                        hT[:, :, tb * 128:(tb + 1) * 128],
                        h_nat[:, tb, :],
                        transpose=True,
                    )

                out_sb = osbp.tile([128, NTB, D], BF16, name=f"out_sb{tci}",
                                   tag="out_sb")

                # Phase 1: all matmuls of the chunk (dense PE stream)
                kk, tt = [], []
                for e in range(NE):
                    es = slice(e * 128, (e + 1) * 128)
                    k_ps = mmps.tile([128, TC], F32, name=f"k{tci}_{e}", tag="k")
                    th_ps = mmps.tile([128, TC], F32, name=f"th{tci}_{e}", tag="th")
                    for d in range(ND):
                        nc.tensor.matmul(k_ps, wz_sb[:, d, es], hT[:, d, :],
                                         start=(d == 0), stop=(d == ND - 1))
                    for d in range(ND):
                        nc.tensor.matmul(th_ps, wh_sb[:, d, es], hT[:, d, :],
                                         start=(d == 0), stop=(d == ND - 1))
                    kk.append(k_ps)
                    tt.append(th_ps)

                # Phase 2: pointwise + scan per e-tile
                scans, hbs = [], []
                for e in range(NE):
                    k_ps, th_ps = kk[e], tt[e]
                    # a = sigmoid(-k-bz); z = sigmoid(k+bz); s = sigmoid(th+bh)
                    a_t = ew.tile([128, TC], F32, name=f"a{tci}_{e}", tag="a")
                    z_t = ew.tile([128, TC], F32, name=f"z{tci}_{e}", tag="z")
                    s_t = ew.tile([128, TC], F32, name=f"s{tci}_{e}", tag="s")
                    nc.scalar.activation(s_t, th_ps, AF.Sigmoid,
                                         bias=bh_sb[:, e:e + 1])
                    nc.scalar.activation(z_t, k_ps, AF.Sigmoid,
                                         bias=bz_sb[:, e:e + 1])
                    nc.scalar.activation(a_t, k_ps, AF.Sigmoid,
                                         bias=negbz[:, e:e + 1], scale=-1.0)
                    # g = max(th + bh + 0.5, s)
                    g_t = ew.tile([128, TC], F32, name=f"g{tci}_{e}", tag="g")
                    nc.vector.scalar_tensor_tensor(g_t, th_ps, bh05[:, e:e + 1],
                                                   s_t, op0=OP.add, op1=OP.max)
                    # b = z * g
                    b_t = ew.tile([128, TC], F32, name=f"b{tci}_{e}", tag="b")
                    nc.gpsimd.tensor_tensor(b_t, z_t, g_t, OP.mult)
                    # h[t] = a[t]*h[t-1] + b[t]
                    h_sc = hscp.tile([128, TC], F32, name=f"hsc{tci}_{e}",
                                     tag=f"hsc{e}")
                    init = 0.0 if tci == 0 else prev_h[e][:, TC - 1:TC]
                    nc.vector.tensor_tensor_scan(h_sc, a_t, b_t, init,
                                                 OP.mult, OP.add)
                    prev_h[e] = h_sc
                    scans.append(h_sc)
                    # cast to bf16 for the xbar output transpose
                    hb = ew.tile([128, TC], BF16, name=f"hb{tci}_{e}", tag="hb")
                    nc.scalar.copy(hb, h_sc)
                    hbs.append(hb)

                # Phase 3: output transpose via DMA xbar ([e,t] -> [t,e], bf16)
                for e in range(NE):
                    es = slice(e * 128, (e + 1) * 128)
                    nc.sync.dma_start(out_sb[:, :, es], hbs[e], transpose=True)

                # ---- store chunk (cast bf16 -> fp32 during SWDGE DMA) ----
                dst = bass.AP(
                    tensor=out_d.tensor,
                    offset=out_d.offset + tci * TC * D,
                    ap=[[D, 128], [128 * D, NTB], [1, D]],
                )
                nc.gpsimd.dma_start(dst, out_sb)

    nc.compile()
    return nc


_nc_cache = None


def _get_program():
    global _nc_cache
    if _nc_cache is None:
        _nc_cache = build_program()
    return _nc_cache


def _make_in_maps(h_prev_layer, W_z, b_z, W_h, b_h):
    wzT = np.ascontiguousarray(W_z.T.astype(np.float32))
    whT = np.ascontiguousarray(W_h.T.astype(np.float32))
    bz8 = np.ascontiguousarray(b_z.reshape(NE, 128).T.astype(np.float32))
    bh8 = np.ascontiguousarray(b_h.reshape(NE, 128).T.astype(np.float32))
    return [
        {
            "h": np.ascontiguousarray(h_prev_layer[i].astype(np.float32)),
            "wzT": wzT, "whT": whT, "bz": bz8, "bh": bh8,
        }
        for i in range(B)
    ]


def run(inputs, trace=False, **kw):
    nc = _get_program()
    in_maps = _make_in_maps(**inputs)
    res = run_bass_kernel_spmd(nc, in_maps, core_ids=list(range(NC_CORES)),
                               trace=trace, **kw)
    out = np.stack([res.results[i]["out"] for i in range(NC_CORES)], axis=0)
    return out, res


def kernel(h_prev_layer, W_z, b_z, W_h, b_h):
    out, _ = run(dict(h_prev_layer=h_prev_layer, W_z=W_z, b_z=b_z,
                      W_h=W_h, b_h=b_h))
    return out


# revision 12
# speedup vs baseline: 1.0375x; 1.0375x over previous
"""MinGRU Trainium2 kernel.

Problem: nn_MinGRU (B=8, T=4096, D=1024, fp32)
    k       = h @ W_z.T + b_z
    th      = h @ W_h.T + b_h
    log-space parallel scan of  h[t] = (1-z[t]) * h[t-1] + z[t]*g(th[t])
with z = sigmoid(k), g(x) = x+0.5 for x>=0 else sigmoid(x).

We compute the mathematically identical linear-space recurrence directly:
    a[t] = sigmoid(-k[t])          (= 1 - z)
    b[t] = sigmoid(k[t]) * g(th[t]),   g(x) = max(x + 0.5, sigmoid(x))
    h[t] = a[t] * h[t-1] + b[t]    (VectorE tensor_tensor_scan, fp32 state)
This is numerically stable since a in (0,1), so it matches the reference's
log-space computation to fp32 rounding.

Sharding: data-parallel over batch — core i gets sample i ([T, D]).
Weights replicated, pre-transposed on host to [d, e] layout (matmul lhsT).

Per-core layout: channels e on partitions, time t along the free dim so the
scan runs along free. h is loaded with a casting DMA (fp32->bf16) and
transposed to [d, t] via the DMA xbar; matmuls run in bf16 (fp32 PSUM
accumulate); everything after the matmul is fp32. Output is transposed back
to [t, e] with PE transposes.
"""

import numpy as np
import concourse.bass as bass
import concourse.bacc as bacc
import concourse.mybir as mybir
import concourse.tile as tile
from concourse.bass_utils import run_bass_kernel_spmd

F32 = mybir.dt.float32
BF16 = mybir.dt.bfloat16
AF = mybir.ActivationFunctionType
OP = mybir.AluOpType

B, T, D = 8, 4096, 1024
NC_CORES = 8
TC = 512                 # time chunk (free dim of one PSUM bank in fp32)
NCHUNK = T // TC         # 8
NE = D // 128            # 8 e-tiles (output channel blocks)
ND = D // 128            # 8 d-tiles (contraction blocks)
NTB = TC // 128          # 4 t-blocks per chunk


def build_program():
    nc = bacc.Bacc("TRN2", target_bir_lowering=False, debug=False)
    h_d = nc.dram_tensor("h", [T, D], F32, kind="ExternalInput").ap()
    wzT_d = nc.dram_tensor("wzT", [D, D], F32, kind="ExternalInput").ap()
    whT_d = nc.dram_tensor("whT", [D, D], F32, kind="ExternalInput").ap()
    bz_d = nc.dram_tensor("bz", [128, NE], F32, kind="ExternalInput").ap()
    bh_d = nc.dram_tensor("bh", [128, NE], F32, kind="ExternalInput").ap()
    out_d = nc.dram_tensor("out", [T, D], F32, kind="ExternalOutput").ap()

    with tile.TileContext(nc) as tc:
        import contextlib
        with contextlib.ExitStack() as ctx:
            const = ctx.enter_context(tc.tile_pool(name="const", bufs=1))
            hnatp = ctx.enter_context(tc.tile_pool(name="hnat", bufs=2))
            hTp = ctx.enter_context(tc.tile_pool(name="hT", bufs=2))
            mmps = ctx.enter_context(tc.tile_pool(name="mmps", bufs=4, space="PSUM"))
            ew = ctx.enter_context(tc.tile_pool(name="ew", bufs=2))
            hscp = ctx.enter_context(tc.tile_pool(name="hsc", bufs=2))
            osbp = ctx.enter_context(tc.tile_pool(name="osb", bufs=2))

            # ---- constants ----
            wz_sb = const.tile([128, ND, D], BF16)   # [d_in_tile, d_tile, e]
            wh_sb = const.tile([128, ND, D], BF16)
            # cast fp32->bf16 during DMA (SWDGE)
            nc.gpsimd.dma_start(wz_sb, wzT_d.rearrange("(dt p) e -> p dt e", p=128))
            nc.gpsimd.dma_start(wh_sb, whT_d.rearrange("(dt p) e -> p dt e", p=128))
            bz_sb = const.tile([128, NE], F32)
            bh_sb = const.tile([128, NE], F32)
            nc.sync.dma_start(bz_sb, bz_d)
            nc.sync.dma_start(bh_sb, bh_d)
            negbz = const.tile([128, NE], F32)
            bh05 = const.tile([128, NE], F32)
            nc.gpsimd.tensor_scalar_mul(negbz, bz_sb, -1.0)
            nc.gpsimd.tensor_scalar_add(bh05, bh_sb, 0.5)

            prev_h = [None] * NE
            hT_tiles = {}

            def load_and_transpose_chunk(ci):
                # load h chunk (cast fp32->bf16), natural [t, d] layout, then
                # transpose to [d, t] via the DMA xbar (no compute-engine cost)
                h_nat = hnatp.tile([128, NTB, D], BF16, name=f"h_nat{ci}",
                                   tag="h_nat")
                hsrc = bass.AP(
                    tensor=h_d.tensor,
                    offset=h_d.offset + ci * TC * D,
                    ap=[[D, 128], [128 * D, NTB], [1, D]],
                )
                nc.gpsimd.dma_start(h_nat, hsrc)
                hT = hTp.tile([128, ND, TC], BF16, name=f"hT{ci}", tag="hT")
                for tb in range(NTB):
                    nc.sync.dma_start(
                        hT[:, :, tb * 128:(tb + 1) * 128],
                        h_nat[:, tb, :],
                        transpose=True,
                    )
                hT_tiles[ci] = hT

            load_and_transpose_chunk(0)
            for tci in range(NCHUNK):
                hT = hT_tiles.pop(tci)

                out_sb = osbp.tile([128, NTB, D], BF16, name=f"out_sb{tci}",
                                   tag="out_sb")

                # Phase 1: all matmuls of the chunk (dense PE stream)
                kk, tt = [], []
                for e in range(NE):
                    es = slice(e * 128, (e + 1) * 128)
                    k_ps = mmps.tile([128, TC], F32, name=f"k{tci}_{e}", tag="k")
                    th_ps = mmps.tile([128, TC], F32, name=f"th{tci}_{e}", tag="th")
                    for d in range(ND):
                        nc.tensor.matmul(k_ps, wz_sb[:, d, es], hT[:, d, :],
                                         start=(d == 0), stop=(d == ND - 1))
                    for d in range(ND):
                        nc.tensor.matmul(th_ps, wh_sb[:, d, es], hT[:, d, :],
                                         start=(d == 0), stop=(d == ND - 1))
                    kk.append(k_ps)
                    tt.append(th_ps)

                # Phase 2: pointwise + scan per e-tile
                scans, hbs = [], []
                for e in range(NE):
                    k_ps, th_ps = kk[e], tt[e]
                    # a = sigmoid(-k-bz); z = sigmoid(k+bz); s = sigmoid(th+bh)
                    a_t = ew.tile([128, TC], F32, name=f"a{tci}_{e}", tag="a")
                    z_t = ew.tile([128, TC], F32, name=f"z{tci}_{e}", tag="z")
                    s_t = ew.tile([128, TC], F32, name=f"s{tci}_{e}", tag="s")
                    nc.scalar.activation(s_t, th_ps, AF.Sigmoid,
                                         bias=bh_sb[:, e:e + 1])
                    nc.scalar.activation(z_t, k_ps, AF.Sigmoid,
                                         bias=bz_sb[:, e:e + 1])
                    nc.scalar.activation(a_t, k_ps, AF.Sigmoid,
                                         bias=negbz[:, e:e + 1], scale=-1.0)
                    # g = max(th + bh + 0.5, s)
                    g_t = ew.tile([128, TC], F32, name=f"g{tci}_{e}", tag="g")
                    nc.vector.scalar_tensor_tensor(g_t, th_ps, bh05[:, e:e + 1],
                                                   s_t, op0=OP.add, op1=OP.max)
                    # b = z * g
                    b_t = ew.tile([128, TC], F32, name=f"b{tci}_{e}", tag="b")
                    nc.gpsimd.tensor_tensor(b_t, z_t, g_t, OP.mult)
                    # h[t] = a[t]*h[t-1] + b[t]
                    h_sc = hscp.tile([128, TC], F32, name=f"hsc{tci}_{e}",
                                     tag=f"hsc{e}")
                    init = 0.0 if tci == 0 else prev_h[e][:, TC - 1:TC]
                    nc.vector.tensor_tensor_scan(h_sc, a_t, b_t, init,
                                                 OP.mult, OP.add)
                    prev_h[e] = h_sc
                    scans.append(h_sc)
                    # cast to bf16 for the xbar output transpose
                    hb = ew.tile([128, TC], BF16, name=f"hb{tci}_{e}", tag="hb")
                    nc.scalar.copy(hb, h_sc)
                    hbs.append(hb)

                # Prefetch next chunk's input + xbar transposes so they sit
                # ahead of this chunk's output transposes in the SP ring.
                if tci + 1 < NCHUNK:
                    load_and_transpose_chunk(tci + 1)

                # Phase 3: output transpose via DMA xbar ([e,t] -> [t,e], bf16)
                for e in range(NE):
                    es = slice(e * 128, (e + 1) * 128)
                    nc.sync.dma_start(out_sb[:, :, es], hbs[e], transpose=True)

                # ---- store chunk (cast bf16 -> fp32 during SWDGE DMA) ----
                dst = bass.AP(
                    tensor=out_d.tensor,
                    offset=out_d.offset + tci * TC * D,
                    ap=[[D, 128], [128 * D, NTB], [1, D]],
                )
                nc.gpsimd.dma_start(dst, out_sb)

    nc.compile()
    return nc


_nc_cache = None


def _get_program():
    global _nc_cache
    if _nc_cache is None:
        _nc_cache = build_program()
    return _nc_cache


def _make_in_maps(h_prev_layer, W_z, b_z, W_h, b_h):
    wzT = np.ascontiguousarray(W_z.T.astype(np.float32))
    whT = np.ascontiguousarray(W_h.T.astype(np.float32))
    bz8 = np.ascontiguousarray(b_z.reshape(NE, 128).T.astype(np.float32))
    bh8 = np.ascontiguousarray(b_h.reshape(NE, 128).T.astype(np.float32))
    return [
        {
            "h": np.ascontiguousarray(h_prev_layer[i].astype(np.float32)),
            "wzT": wzT, "whT": whT, "bz": bz8, "bh": bh8,
        }
        for i in range(B)
    ]


def run(inputs, trace=False, **kw):
    nc = _get_program()
    in_maps = _make_in_maps(**inputs)
    res = run_bass_kernel_spmd(nc, in_maps, core_ids=list(range(NC_CORES)),
                               trace=trace, **kw)
    out = np.stack([res.results[i]["out"] for i in range(NC_CORES)], axis=0)
    return out, res


def kernel(h_prev_layer, W_z, b_z, W_h, b_h):
    out, _ = run(dict(h_prev_layer=h_prev_layer, W_z=W_z, b_z=b_z,
                      W_h=W_h, b_h=b_h))
    return out
